# revision 34
# baseline (speedup 1.0000x reference)
"""Bass/Trainium2 kernel for nn_Expert_WNO2d (8-expert gated WaveConv2d mixture).

Math: the reference is linear in x. Every expert passes the fine Haar detail
levels (1..3) through unchanged and only channel-mixes the coarsest (level-4)
approximation + detail coefficients. With gate slots s weighting experts
PERM = (0,1,2,3,4,5,4,5), the output collapses to (with G = sum_s lambda)

    y[b] = G[b]*x[b] + rep8( 0.125*idwt4(sum_e geff[b,e]*(W_e . c4[b]))
                             - (G[b]/64)*s8[b] )

Implementation notes (per core: 4 samples, data-parallel over B=32/8 cores):
 - host premultiplies x by G (bf16); gate tile ge carries geff/G plus the
   1/256 analysis+synthesis constant and the fp8 weight descale.
 - s8 = 8x8 block sums via two bf16 DVE folds + XY reduces per sub-tile.
 - coefficients cf = sign butterfly of s8; cc = cf * ge (bf16, 2x DVE).
 - expert weights replicated in fp8e4 scaled 2^12; matmuls pack two modes
   per [128,128] stationary load (FWL fast weight load) with N=8 moving
   columns, accumulating 3 expert-pair chunks in PSUM.
 - synthesis: band +- combos (negations on ScalarE), scatter to adj[(u,v)],
   adj_w = adj + (-1/64)s8 expanded over w; final y = x + rep8(adj) is a
   plain bf16 tensor_tensor add at DVE 2x, streaming y out per sub-tile,
   chained per row-tile so rt0's stores overlap rt1's synthesis.
"""

import numpy as np

import concourse.bacc as bacc
import concourse.mybir as mybir
import concourse.tile as tile

N_CORES = 8
B, C, S = 32, 64, 64
BL = B // N_CORES          # samples per core = 4
NE = 6                     # live experts
K_SHIFT = 12               # fp8 weight upscale 2^k
f32 = mybir.dt.float32
bf16 = mybir.dt.bfloat16
fp8 = mybir.dt.float8e4
ALU = mybir.AluOpType


def _build_nc():
    nc = bacc.Bacc()
    xw = nc.declare_dram_parameter("xw", [2, 128, 4096], bf16, isOutput=False)
    wt = nc.declare_dram_parameter("wt", [4, 128, 3072], fp8, isOutput=False)
    ge = nc.declare_dram_parameter("ge", [2, 128, 384], bf16, isOutput=False)
    yw = nc.declare_dram_parameter("yw", [2, 128, 4096], bf16, isOutput=True)

    with tile.TileContext(nc) as tc:
        with (
            tc.tile_pool(name="xp", bufs=8) as xp,
            tc.tile_pool(name="yp", bufs=8) as yp,
            tc.tile_pool(name="wp", bufs=4) as wp,
            tc.tile_pool(name="sp", bufs=2) as sp,
            tc.tile_pool(name="fp", bufs=4) as fpl,
            tc.tile_pool(name="cp", bufs=2) as cp,
            tc.tile_pool(name="up", bufs=2) as up,
            tc.tile_pool(name="ps", bufs=4, space="PSUM") as psp,
        ):
            xs = [[], []]
            for rt in range(2):
                for c in range(4):
                    xt = xp.tile([128, 1024], bf16, tag="xs", name=f"x{rt}{c}")
                    if rt == 0 and c == 0:
                        # split first transfer: completes sooner after the
                        # trailing DMA queues come up
                        nc.sync.dma_start(out=xt[:, 0:512], in_=xw[rt, :, 0:512])
                        nc.sync.dma_start(out=xt[:, 512:1024], in_=xw[rt, :, 512:1024])
                    else:
                        nc.sync.dma_start(out=xt[:, :], in_=xw[rt, :, 1024 * c:1024 * (c + 1)])
                    xs[rt].append(xt)
            ge_s = []
            for rt in range(2):
                gee = sp.tile([128, 384], bf16, tag="ge", name=f"ge{rt}")
                nc.sync.dma_start(out=gee[:, :], in_=ge[rt, :, :])
                ge_s.append(gee)
            wt_s = []
            for band in range(4):
                w = wp.tile([128, 3072], fp8, tag="wt", name=f"w{band}")
                nc.sync.dma_start(out=w[:, :], in_=wt[band, :, :])
                wt_s.append(w)

            # ---- block sums: per sub-tile, 2 bf16 folds + per-ub XY reduce ----
            s8 = []
            for rt in range(2):
                s8t = sp.tile([128, 64], f32, tag="s8", name=f"s8{rt}")
                s8.append(s8t)
            for rt in range(2):
                for c in range(4):
                    xr = xs[rt][c][:, :].rearrange("p (r v t) -> p r v t", r=16, v=8, t=8)
                    ft1 = fpl.tile([128, 512], bf16, tag="f1", name=f"f1_{rt}{c}")
                    f1r = ft1[:, :].rearrange("p (r v t) -> p r v t", r=16, v=8, t=4)
                    if rt == 0 and c == 0:
                        nc.vector.tensor_add(f1r[:, 0:8], xr[:, 0:8, :, 0:4], xr[:, 0:8, :, 4:8])
                        nc.vector.tensor_add(f1r[:, 8:16], xr[:, 8:16, :, 0:4], xr[:, 8:16, :, 4:8])
                    else:
                        nc.vector.tensor_add(f1r, xr[:, :, :, 0:4], xr[:, :, :, 4:8])
                    ft2 = fpl.tile([128, 256], bf16, tag="f2", name=f"f2_{rt}{c}")
                    f2r = ft2[:, :].rearrange("p (r v t) -> p r v t", r=16, v=8, t=2)
                    nc.vector.tensor_add(f2r, f1r[:, :, :, 0:2], f1r[:, :, :, 2:4])
                    f2u = ft2[:, :].rearrange("p (ub dr v t) -> p ub v dr t", ub=2, dr=8, v=8, t=2)
                    for ub in range(2):
                        nc.vector.tensor_reduce(
                            out=s8[rt][:, :].rearrange("p (u v) -> p u v", u=8, v=8)[:, 2 * c + ub, :],
                            in_=f2u[:, ub],
                            axis=mybir.AxisListType.XY, op=ALU.add,
                        )

            # ---- level-4 Haar analysis (scales folded into ge) ----
            cf = []
            for rt in range(2):
                s8v = s8[rt][:, :].rearrange("p (u v) -> p u v", u=8, v=8)
                t2 = up.tile([128, 64], f32, tag="t2", name=f"t2_{rt}")
                t2v = t2[:, :].rearrange("p (pm u y) -> p pm u y", pm=2, u=8, y=4)
                nc.vector.tensor_add(t2v[:, 0], s8v[:, :, 0:8:2], s8v[:, :, 1:8:2])
                nc.vector.tensor_sub(t2v[:, 1], s8v[:, :, 0:8:2], s8v[:, :, 1:8:2])
                cft = sp.tile([128, 64], bf16, tag="cf", name=f"cf{rt}")
                cfv = cft[:, :].rearrange("p (bd x y) -> p bd x y", bd=4, x=4, y=4)
                nc.vector.tensor_add(cfv[:, 0], t2v[:, 0, 0:8:2, :], t2v[:, 0, 1:8:2, :])
                nc.vector.tensor_sub(cfv[:, 1], t2v[:, 0, 0:8:2, :], t2v[:, 0, 1:8:2, :])
                nc.vector.tensor_add(cfv[:, 2], t2v[:, 1, 0:8:2, :], t2v[:, 1, 1:8:2, :])
                nc.vector.tensor_sub(cfv[:, 3], t2v[:, 1, 0:8:2, :], t2v[:, 1, 1:8:2, :])
                cf.append(cft)

            # ---- per row-tile: cc, matmuls, synthesis, final: rt0's back-end
            # overlaps rt1's coefficient prep and matmuls ----
            for rt in range(2):
                cc = cp.tile([128, 384], bf16, tag="cc", name=f"cc{rt}")
                for bh in range(2):
                    b = rt * 2 + bh
                    for el in range(2):
                        nc.vector.tensor_tensor(
                            out=cc[el * 64:(el + 1) * 64, :]
                                .rearrange("p (ch bb m) -> p ch bb m", ch=3, bb=2, m=64)[:, :, bh],
                            in0=cf[rt][bh * 64:(bh + 1) * 64, :]
                                .rearrange("p (o m) -> p o m", o=1)
                                .broadcast_to([64, 3, 64]),
                            in1=ge_s[rt][bh * 64:(bh + 1) * 64, :]
                                .rearrange("p (el ch m) -> p el ch m", el=2, ch=3)[:, el],
                            op=ALU.mult,
                        )
                ccv = cc[:, :].rearrange("p (ch bb m) -> p ch m bb", ch=3, bb=2, m=64)
                pbp = [psp.tile([128, 32], f32, tag="pb", name=f"pb{rt}{i}")
                       for i in range(4)]
                for band in range(4):
                    for j in range(8):
                        for ch in range(3):
                            nc.tensor.matmul(
                                out=pbp[band][:, j * 4:(j + 1) * 4],
                                lhsT=wt_s[band][:, (j * 3 + ch) * 128:(j * 3 + ch) * 128 + 128],
                                rhs=ccv[:, ch, band * 16 + 2 * j:band * 16 + 2 * j + 2, :],
                                start=(ch == 0), stop=(ch == 2),
                            )
                s1 = up.tile([128, 32], f32, tag="s1", name=f"s1_{rt}")
                s3 = up.tile([128, 32], f32, tag="s3", name=f"s3_{rt}")
                nc.scalar.copy(s1[:, :], pbp[1][:, :])
                nc.scalar.copy(s3[:, :], pbp[3][:, :])
                uAB = up.tile([128, 64], f32, tag="uAB", name=f"uAB{rt}")
                uCD = up.tile([128, 128], f32, tag="uCD", name=f"uCD{rt}")
                nc.vector.tensor_add(uAB[:, 0:32], pbp[0][:, :], s1[:, :])
                nc.vector.tensor_sub(uAB[:, 32:64], pbp[0][:, :], s1[:, :])
                nc.vector.tensor_add(uCD[:, 0:32], pbp[2][:, :], s3[:, :])
                nc.vector.tensor_sub(uCD[:, 32:64], pbp[2][:, :], s3[:, :])
                nc.scalar.mul(uCD[:, 64:96], uCD[:, 0:32], -1.0)
                nc.scalar.mul(uCD[:, 96:128], uCD[:, 32:64], -1.0)
                att = sp.tile([128, 64], f32, tag="at", name=f"at{rt}")
                for bh in range(2):
                    b = rt * 2 + bh
                    for h in range(2):
                        for di in range(2):
                            ov = att[bh * 64:(bh + 1) * 64, :].rearrange(
                                "p (x di yp hh dj) -> p di hh x yp dj",
                                x=4, di=2, yp=2, hh=2, dj=2)[:, di, h]
                            in0 = uAB[h * 64:(h + 1) * 64, :].rearrange(
                                "p (k xx yp s) -> p k xx yp s", k=2, xx=4, yp=2, s=4)[
                                :, di, :, :, h * 2 + bh:h * 2 + bh + 1].broadcast_to([64, 4, 2, 2])
                            in1 = uCD[h * 64:(h + 1) * 64, :].rearrange(
                                "p (pm k xx yp s) -> p k xx yp pm s", pm=2, k=2, xx=4, yp=2, s=4)[
                                :, di, :, :, :, h * 2 + bh]
                            nc.vector.tensor_add(ov, in0, in1)
                aw = sp.tile([128, 512], bf16, tag="aw", name=f"aw{rt}")
                nc.vector.scalar_tensor_tensor(
                    out=aw[:, :].rearrange("p (uv t) -> p uv t", uv=64, t=8),
                    in0=s8[rt][:, :].rearrange("p (uv o) -> p uv o", uv=64, o=1)
                        .broadcast_to([128, 64, 8]),
                    scalar=-1.0 / 64.0,
                    in1=att[:, :].rearrange("p (uv o) -> p uv o", uv=64, o=1)
                        .broadcast_to([128, 64, 8]),
                    op0=ALU.mult, op1=ALU.add,
                )
                awv = aw[:, :].rearrange("p (u o vt) -> p u o vt", u=8, o=1, vt=64)
                for c in range(4):
                    ys = yp.tile([128, 1024], bf16, tag="ys", name=f"y{rt}{c}")
                    nc.vector.tensor_add(
                        ys[:, :].rearrange("p (ub dr vt) -> p ub dr vt", ub=2, dr=8, vt=64),
                        xs[rt][c][:, :].rearrange("p (ub dr vt) -> p ub dr vt", ub=2, dr=8, vt=64),
                        awv[:, 2 * c:2 * c + 2].broadcast_to([128, 2, 8, 64]),
                    )
                    nc.sync.dma_start(out=yw[rt, :, 1024 * c:1024 * (c + 1)], in_=ys[:, :])
    nc.compile()
    return nc


_NC = None


def _get_nc():
    global _NC
    if _NC is None:
        _NC = _build_nc()
    return _NC


def _pack_weights(WL, WH):
    import ml_dtypes
    Wall = np.empty((4, NE, C, C, 4, 4), np.float32)
    Wall[0] = WL[:NE]
    for k in range(3):
        Wall[k + 1] = WH[:NE, k]
    Wall *= float(2 ** K_SHIFT)
    W6 = Wall.reshape(4, 3, 2, C, C, 16)            # band, ch, el, i, o, mode
    T = W6.transpose(0, 2, 3, 5, 1, 4)              # band, el, i, mode, ch, o
    T = T.reshape(4, 2, C, 8, 2, 3, C)              # band el i j h ch o
    T = T.transpose(0, 1, 2, 3, 5, 4, 6)            # band el i j ch h o
    return np.ascontiguousarray(T.reshape(4, 128, 3072)).astype(ml_dtypes.float8_e4m3fn)


def _pack_gates(lambda_):
    """x is host-premultiplied by G; ge carries geff/G with the 1/256
    analysis+synthesis constant and the fp8 weight descale folded in."""
    import ml_dtypes
    lam = lambda_.reshape(B, 8).astype(np.float32)
    G = lam.sum(1)
    geff = lam[:, :6].copy()
    geff[:, 4] += lam[:, 6]
    geff[:, 5] += lam[:, 7]
    gsc = geff * (0.0625 * 0.0625 * (2.0 ** -K_SHIFT)) / G[:, None]
    ge = np.zeros((B, 2, 3, 64), np.float32)
    for el in range(2):
        for ch in range(3):
            ge[:, el, ch, :] = gsc[:, ch * 2 + el][:, None]
    return G, ge.reshape(B, 384).astype(ml_dtypes.bfloat16)


def kernel(x, lambda_, WL, WH):
    import ml_dtypes
    from concourse.bass_utils import run_bass_kernel_spmd

    nc = _get_nc()
    wt = _pack_weights(np.asarray(WL, np.float32), np.asarray(WH, np.float32))
    G, ge = _pack_gates(np.asarray(lambda_, np.float32))
    xb = (np.asarray(x, np.float32) * G[:, None, None, None]).astype(ml_dtypes.bfloat16)

    in_maps = []
    for k in range(N_CORES):
        xl = np.ascontiguousarray(xb[k * BL:(k + 1) * BL].reshape(2, 128, 4096))
        gel = np.repeat(ge[k * BL:(k + 1) * BL], C, axis=0).reshape(2, 128, 384)
        in_maps.append({"xw": xl,
                        "wt": wt,
                        "ge": np.ascontiguousarray(gel)})

    res = run_bass_kernel_spmd(nc, in_maps, list(range(N_CORES)))
    out = np.empty((B, C, S, S), np.float32)
    for k in range(N_CORES):
        out[k * BL:(k + 1) * BL] = res.results[k]["yw"].astype(np.float32).reshape(BL, C, S, S)
    return out
